# revision 35
# baseline (speedup 1.0000x reference)
"""Causal multi-head attention (dense transformer block) on 8 Trainium2 cores.

Problem: x[4, 2048, 1024], 16 heads, head_dim 64, causal softmax attention
with QKV + output projections (torch Linear layout weights).

Sharding: 8 cores = 4 batches x 2 head-groups (8 heads each).  Each core
computes QKV projection for its 8 heads, attention, and its partial output
projection (row-parallel over w_out).  Host sums the two partials per batch
and adds b_out.

All device layouts are "transposed" so no on-device transposes are needed:
  - x is fed as xT [d, s]; Q^T/K^T are produced as [head_dim, s]
  - scores are computed as S^T [k, q]; softmax runs along partitions via
    ones-matmul column sums; O is accumulated directly as O^T [e_loc, q],
    which is exactly the lhsT the output projection needs.
Matmul inputs are bf16 (PSUM accumulation is fp32); everything else fp32.

Schedule: attention processes one head-pair at a time (psov needs only one
PSUM bank, freeing a second bank so projection/output chains double-buffer
through the shared "psx" pool).  Only the minimum QKV slice runs up front
(KT/QT(hp0) + V0/V1); the remaining V/QT/KT chunks and the output
projection live in a deadline-ordered filler queue drained inside the
attention i-loops, so each writer is traced before its readers (Tile
tracks RAW deps by trace order) while the PE interleaves chains into exp
gaps.  Within the i-loop, scores for tile i+1 are issued before AV(i) so
ACT's exp stream never starves.

Note: PSUM accumulation-group starts (start=True) clear has_written for
the whole bank, so the per-(j,hp) denominator accumulator keeps a single
group (rows 0/32) — interleaving a second group's start on the same bank
corrupts or races.
"""

import sys

sys.path.insert(0, "/opt/trn_rl_repo")

import numpy as np
import ml_dtypes

import concourse.bass as bass
import concourse.mybir as mybir
import concourse.tile as tile
from concourse import bacc
from concourse import bass_utils
from concourse.masks import make_upper_triangular

F32 = mybir.dt.float32
BF16 = mybir.dt.bfloat16
EXP = mybir.ActivationFunctionType.Exp

B, S, D = 4, 2048, 1024
HTOT, HD = 16, 64
NCORES = 8
HLOC = HTOT // 2          # heads per core
ELOC = HLOC * HD          # 512 local embedding width
NHP = HLOC // 2           # 4 head pairs
QC = 512                  # q-chunk width
NQC = S // QC             # 4
NKT = S // 128            # 16 k tiles over sequence
NDT = D // 128            # 8 k tiles over model dim
SCALE = 1.0 / float(np.sqrt(HD))

_CACHE = {}


def _build_nc():
    nc = bacc.Bacc("TRN2", target_bir_lowering=False, debug=False)

    xT = nc.dram_tensor("xT", [D, S], BF16, kind="ExternalInput")
    wqT = nc.dram_tensor("wqT", [D, ELOC], BF16, kind="ExternalInput")
    wkT = nc.dram_tensor("wkT", [D, ELOC], BF16, kind="ExternalInput")
    wvT = nc.dram_tensor("wvT", [D, ELOC], BF16, kind="ExternalInput")
    woT = nc.dram_tensor("woT", [ELOC, D], BF16, kind="ExternalInput")
    bqk = nc.dram_tensor("bqk", [128, 2, NHP], F32, kind="ExternalInput")
    bvb = nc.dram_tensor("bvb", [128, ELOC], F32, kind="ExternalInput")
    outp = nc.dram_tensor("outp", [S, D], F32, kind="ExternalOutput")

    with tile.TileContext(nc) as tc:
        with tc.tile_pool(name="const", bufs=1) as constp, \
             tc.tile_pool(name="wpool", bufs=1) as wp, \
             tc.tile_pool(name="qkv", bufs=1) as qkvp, \
             tc.tile_pool(name="xt", bufs=1) as xtp, \
             tc.tile_pool(name="pt", bufs=12) as ptp, \
             tc.tile_pool(name="otn", bufs=8) as otnp, \
             tc.tile_pool(name="dr", bufs=4) as drp, \
             tc.tile_pool(name="osb", bufs=6) as osbp, \
             tc.tile_pool(name="pss", bufs=2, space="PSUM") as pss, \
             tc.tile_pool(name="psov", bufs=1, space="PSUM") as psov, \
             tc.tile_pool(name="psd", bufs=1, space="PSUM") as psd, \
             tc.tile_pool(name="psx", bufs=2, space="PSUM") as psx:

            # ---- constants ----
            trimask = constp.tile([128, 128], BF16, name="trimask")
            make_upper_triangular(nc, trimask[:], val=1.0, diag=True)
            ones_a = constp.tile([128, 1], BF16, name="ones_a")
            nc.gpsimd.memset(ones_a[:], 1.0)
            ones_b = constp.tile([128, 2], BF16, name="ones_b")
            nc.gpsimd.memset(ones_b[:], 0.0)
            nc.gpsimd.memset(ones_b[:, 0:1], 1.0)
            bc_x = constp.tile([98, 128], BF16, name="bc_x")
            nc.gpsimd.memset(bc_x[:], 0.0)
            nc.gpsimd.memset(bc_x[0:1, 0:64], 1.0)
            nc.gpsimd.memset(bc_x[32:33, 64:128], 1.0)

            bqk_sb = constp.tile([128, 2, NHP], F32, name="bqk_sb")
            nc.sync.dma_start(bqk_sb[:], bqk[:])
            bvb_sb = constp.tile([128, ELOC], F32, name="bvb_sb")
            nc.sync.dma_start(bvb_sb[:], bvb[:])

            # ---- weight + xT DMAs, ordered by first use:
            # wk (KT chunk0), xT (everything), wv (V0), wq (QT c3), wo ----
            wk_sb = []
            for kt in range(NDT):
                t = wp.tile([128, ELOC], BF16, name=f"wk{kt}")
                nc.sync.dma_start(t[:], wkT[128 * kt:128 * (kt + 1), :])
                wk_sb.append(t)
            xts = []
            for kt in range(NDT):
                t = xtp.tile([128, S], BF16, name=f"xt{kt}")
                nc.sync.dma_start(t[:], xT[128 * kt:128 * (kt + 1), :])
                xts.append(t)
            wv_sb = []
            for kt in range(NDT):
                t = wp.tile([128, ELOC], BF16, name=f"wv{kt}")
                nc.sync.dma_start(t[:], wvT[128 * kt:128 * (kt + 1), :])
                wv_sb.append(t)
            wq_sb = []
            for kt in range(NDT):
                t = wp.tile([128, ELOC], BF16, name=f"wq{kt}")
                nc.sync.dma_start(t[:], wqT[128 * kt:128 * (kt + 1), :])
                wq_sb.append(t)
            wo_sb = []
            for hp in range(NHP):
                t = wp.tile([128, D], BF16, name=f"wo{hp}")
                nc.sync.dma_start(t[:], woT[128 * hp:128 * (hp + 1), :])
                wo_sb.append(t)

            # ---- QKV tile holders ----
            QT, KT = [], []
            for hp in range(NHP):
                QT.append(qkvp.tile([128, S], BF16, name=f"qt{hp}"))
                KT.append(qkvp.tile([128, S], BF16, name=f"kt{hp}"))
            V = [qkvp.tile([128, ELOC], BF16, name=f"v{st}") for st in range(NKT)]

            def make_v(st):
                ps = psx.tile([128, ELOC], F32, tag="psx", name="ps_v")
                for kt in range(NDT):
                    nc.tensor.matmul(
                        ps[:],
                        lhsT=xts[kt][:, 128 * st:128 * (st + 1)],
                        rhs=wv_sb[kt][:],
                        start=(kt == 0), stop=(kt == NDT - 1))
                nc.vector.tensor_add(V[st][:], ps[:], bvb_sb[:])

            def make_qk_chunk(which, hp, c):
                dst, wsb, col = ((QT, wq_sb, 0) if which == "q"
                                 else (KT, wk_sb, 1))
                ps = psx.tile([128, QC], F32, tag="psx", name="ps_qk")
                for kt in range(NDT):
                    nc.tensor.matmul(
                        ps[:],
                        lhsT=wsb[kt][:, 128 * hp:128 * (hp + 1)],
                        rhs=xts[kt][:, QC * c:QC * (c + 1)],
                        start=(kt == 0), stop=(kt == NDT - 1))
                nc.vector.tensor_scalar_add(
                    dst[hp][:, QC * c:QC * (c + 1)], ps[:],
                    bqk_sb[:, col, hp:hp + 1])

            # ---- minimal upfront compute: the first attention block
            # (j=3, hp=0) needs KT(hp0) all chunks + QT(hp0) chunk 3;
            # its first AV needs V0/V1 ----
            for c in range(NQC):
                make_qk_chunk("k", 0, c)
            make_qk_chunk("q", 0, 3)
            make_v(0)
            make_v(1)

            # ---- filler queue: each entry is one projection chain (8 MMs
            # + drain).  They are woven into the attention loops at normal
            # priority, in deadline order, so the PE alternates between
            # attention windows and filler chains instead of starving
            # either. ----
            from collections import deque
            fillers = deque()
            for st in range(2, NKT):
                fillers.append(lambda st=st: make_v(st))
            for hp in (1, 2, 3):
                for c in range(NQC):
                    fillers.append(
                        lambda hp=hp, c=c: make_qk_chunk("k", hp, c))
                fillers.append(lambda hp=hp: make_qk_chunk("q", hp, 3))
            for c in (2, 1, 0):
                for hp in (0, 1, 2, 3):
                    fillers.append(
                        lambda hp=hp, c=c: make_qk_chunk("q", hp, c))

            def drain_fillers(n=1):
                for _ in range(min(n, len(fillers))):
                    fillers.popleft()()

            # ---- attention + output projection ----
            tri3 = trimask[:][:, None, :].broadcast_to([128, 2, 128])

            def issue_scores(hp, j, i):
                """Score matmuls for key-tile i of chunk j: 2 MMs, the two
                h2 halves row-pack (lhsT base partitions 0/64)."""
                w = 128 * (i - 4 * j) if i >= 4 * j else 0
                ps_s = pss.tile([128, 2, QC], F32, tag="pss", name="ps_s")
                for h2 in range(2):
                    nc.tensor.matmul(
                        ps_s[:, h2, w:QC],
                        lhsT=KT[hp][64 * h2:64 * (h2 + 1),
                                    128 * i:128 * (i + 1)],
                        rhs=QT[hp][64 * h2:64 * (h2 + 1),
                                   QC * j + w:QC * (j + 1)],
                        start=True, stop=True)
                return ps_s

            for j in (3, 2, 1, 0):
                nkt = 4 * j + 4
                otn_j = {}
                for hp in range(NHP):
                    ps_ot = psov.tile([128, QC], F32, tag="psov",
                                      name="ps_ot")
                    ps_d = psd.tile([128, QC], F32, tag="psd", name="ps_d")
                    if j == 3 and hp == 0:
                        nc.vector.memset(ps_d[:], 1.0)
                    ss = issue_scores(hp, j, 0)
                    pts = {}

                    def issue_av_dens(ii):
                        wd = 128 * (ii - 4 * j) if ii >= 4 * j else 0
                        pd = pts.pop(ii)
                        for h2 in range(2):
                            nc.tensor.matmul(
                                ps_ot[64 * h2:64 * (h2 + 1), wd:QC],
                                lhsT=V[ii][:, 64 * (2 * hp + h2):
                                           64 * (2 * hp + h2 + 1)],
                                rhs=pd[:, h2, wd:QC],
                                start=(ii == 0), stop=(ii == nkt - 1),
                                tile_position=(0, 64 * h2))
                        nc.tensor.matmul(
                            ps_d[0:1, wd:QC],
                            lhsT=ones_a[:], rhs=pd[:, 0, wd:QC],
                            start=(ii == 0), stop=(ii == nkt - 1),
                            tile_position=(0, 0))
                        nc.tensor.matmul(
                            ps_d[32:34, wd:QC],
                            lhsT=ones_b[:], rhs=pd[:, 1, wd:QC],
                            start=(ii == 0), stop=(ii == nkt - 1),
                            tile_position=(0, 32))

                    for i in range(nkt):
                        # keep filler writers traced well ahead of their
                        # readers (V[i] feeds AV(i) two iterations later)
                        drain_fillers(1)
                        w = 128 * (i - 4 * j) if i >= 4 * j else 0
                        last = (i == nkt - 1)
                        pt = ptp.tile([128, 2, QC], BF16, tag="pt",
                                      name="pt")
                        pts[i] = pt
                        nc.scalar.activation(pt[:, :, w:QC], ss[:, :, w:QC],
                                             EXP, scale=SCALE)
                        if i >= 4 * j:
                            nc.vector.tensor_mul(
                                pt[:, :, w:w + 128],
                                pt[:, :, w:w + 128], tri3[:, :, :])
                        if not last:
                            ss = issue_scores(hp, j, i + 1)
                        # AV + denominators lag one iteration: everything
                        # issued here is already dependency-free, so the
                        # PE never waits on the exp just dispatched.
                        if i > 0:
                            issue_av_dens(i - 1)
                            if i % 2 == 0:
                                drain_fillers(1)
                    issue_av_dens(nkt - 1)
                    # normalization: combine the even/odd denominator
                    # accumulators, sanitize + approx reciprocal, then one
                    # broadcast matmul reusing the psd bank.
                    xs = drp.tile([34, QC], F32, name="xs")
                    nc.vector.tensor_scalar_max(xs[:], ps_d[0:34, :],
                                                1e-30)
                    drf = drp.tile([34, QC], F32, name="drf")
                    nc.vector.reciprocal_approx_fast(drf[:], xs[:])
                    dr = drp.tile([34, QC], BF16)
                    with nc.allow_low_precision(reason="denom bf16"):
                        nc.vector.tensor_copy(dr[:], drf[:])
                    nc.tensor.matmul(ps_d[:], lhsT=bc_x[0:34, :],
                                     rhs=dr[:], start=True, stop=True)
                    dbc = drp.tile([128, QC], BF16, name="dbc")
                    nc.vector.tensor_copy(dbc[:], ps_d[:])
                    otn = otnp.tile([128, QC], BF16, tag="otn",
                                    name="otn")
                    nc.vector.tensor_mul(otn[:], ps_ot[:], dbc[:])
                    otn_j[hp] = otn
                # output projection for this q chunk: queued as fillers
                # (it has no downstream consumer besides the final DMA),
                # woven into later blocks' loops.
                def make_outproj(otns, j, m, eo):
                    s0 = QC * j + 128 * m
                    ps_o = psx.tile([128, 512], F32, tag="psx",
                                    name="ps_o")
                    for hp in range(NHP):
                        nc.tensor.matmul(
                            ps_o[:],
                            lhsT=otns[hp][:, 128 * m:128 * (m + 1)],
                            rhs=wo_sb[hp][:, 512 * eo:512 * (eo + 1)],
                            start=(hp == 0), stop=(hp == NHP - 1))
                    osb = osbp.tile([128, 512], F32)
                    nc.vector.tensor_copy(osb[:], ps_o[:])
                    nc.sync.dma_start(
                        outp[s0:s0 + 128, 512 * eo:512 * (eo + 1)],
                        osb[:])

                otns = dict(otn_j)
                for m in range(4):
                    for eo in range(2):
                        fillers.append(
                            lambda otns=otns, j=j, m=m, eo=eo:
                            make_outproj(otns, j, m, eo))
            drain_fillers(len(fillers))

    nc.compile()
    return nc


def _get_nc():
    if "nc" not in _CACHE:
        _CACHE["nc"] = _build_nc()
    return _CACHE["nc"]


def _prep_core_inputs(x, w_qkv, b_qkv, w_out, b, hg):
    r0 = ELOC * hg
    wq = w_qkv[r0:r0 + ELOC, :]
    wk = w_qkv[D + r0:D + r0 + ELOC, :]
    wv = w_qkv[2 * D + r0:2 * D + r0 + ELOC, :]
    bq = b_qkv[r0:r0 + ELOC]
    bk = b_qkv[D + r0:D + r0 + ELOC]
    bv = b_qkv[2 * D + r0:2 * D + r0 + ELOC]

    bf = ml_dtypes.bfloat16
    bqk_arr = np.empty((128, 2, NHP), np.float32)
    bqk_arr[:, 0, :] = bq.reshape(NHP, 128).T
    bqk_arr[:, 1, :] = bk.reshape(NHP, 128).T
    return {
        "xT": np.ascontiguousarray(x[b].T).astype(bf),
        "wqT": np.ascontiguousarray(wq.T).astype(bf),
        "wkT": np.ascontiguousarray(wk.T).astype(bf),
        "wvT": np.ascontiguousarray(wv.T).astype(bf),
        "woT": np.ascontiguousarray(w_out[:, r0:r0 + ELOC].T).astype(bf),
        "bqk": bqk_arr,
        "bvb": np.tile(bv.astype(np.float32)[None, :], (128, 1)),
    }


def kernel(x, w_qkv, b_qkv, w_out, b_out, _trace=False, _trace_kwargs=None):
    x = np.asarray(x, np.float32)
    w_qkv = np.asarray(w_qkv, np.float32)
    b_qkv = np.asarray(b_qkv, np.float32)
    w_out = np.asarray(w_out, np.float32)
    b_out = np.asarray(b_out, np.float32)

    nc = _get_nc()
    in_maps = []
    for core in range(NCORES):
        b, hg = core // 2, core % 2
        in_maps.append(_prep_core_inputs(x, w_qkv, b_qkv, w_out, b, hg))

    kw = {}
    if _trace:
        kw.update(trace=True, **(_trace_kwargs or {}))
    import time
    res = None
    for attempt in range(4):
        try:
            res = bass_utils.run_bass_kernel_spmd(
                nc, in_maps, core_ids=list(range(NCORES)), **kw)
            break
        except Exception:
            if attempt == 3:
                raise
            # Transient axon/NRT device flake: reset the PJRT backend so the
            # retry starts from a clean client, like a fresh process would.
            try:
                import jax
                jax.clear_caches()
                import jax._src.xla_bridge as _xb
                _xb._clear_backends()
            except Exception:
                pass
            time.sleep(5.0 * (attempt + 1))

    out = np.empty((B, S, D), np.float32)
    for b in range(B):
        out[b] = res.results[2 * b]["outp"] + res.results[2 * b + 1]["outp"] \
            + b_out[None, :]
    if _trace:
        return out, res
    return out


# revision 36
# speedup vs baseline: 1.0544x; 1.0544x over previous
"""Causal multi-head attention (dense transformer block) on 8 Trainium2 cores.

Problem: x[4, 2048, 1024], 16 heads, head_dim 64, causal softmax attention
with QKV + output projections (torch Linear layout weights).

Sharding: 8 cores = 4 batches x 2 head-groups (8 heads each).  Each core
computes QKV projection for its 8 heads, attention, and its partial output
projection (row-parallel over w_out).  Host sums the two partials per batch
and adds b_out.

All device layouts are "transposed" so no on-device transposes are needed:
  - x is fed as xT [d, s]; Q^T/K^T are produced as [head_dim, s]
  - scores are computed as S^T [k, q]; softmax runs along partitions via
    ones-matmul column sums; O is accumulated directly as O^T [e_loc, q],
    which is exactly the lhsT the output projection needs.
Matmul inputs are bf16 (PSUM accumulation is fp32); everything else fp32.

Schedule: attention processes one head-pair at a time (psov needs only one
PSUM bank, freeing a second bank so projection/output chains double-buffer
through the shared "psx" pool).  Only the minimum QKV slice runs up front
(KT/QT(hp0) + V0/V1); the remaining V/QT/KT chunks and the output
projection live in a deadline-ordered filler queue drained inside the
attention i-loops, so each writer is traced before its readers (Tile
tracks RAW deps by trace order) while the PE interleaves chains into exp
gaps.  Within the i-loop, scores for tile i+1 are issued before AV(i) so
ACT's exp stream never starves.

Note: PSUM accumulation-group starts (start=True) clear has_written for
the whole bank, so the per-(j,hp) denominator accumulator keeps a single
group (rows 0/32) — interleaving a second group's start on the same bank
corrupts or races.
"""

import sys

sys.path.insert(0, "/opt/trn_rl_repo")

import numpy as np
import ml_dtypes

import concourse.bass as bass
import concourse.mybir as mybir
import concourse.tile as tile
from concourse import bacc
from concourse import bass_utils
from concourse.masks import make_upper_triangular

F32 = mybir.dt.float32
BF16 = mybir.dt.bfloat16
EXP = mybir.ActivationFunctionType.Exp

B, S, D = 4, 2048, 1024
HTOT, HD = 16, 64
NCORES = 8
HLOC = HTOT // 2          # heads per core
ELOC = HLOC * HD          # 512 local embedding width
NHP = HLOC // 2           # 4 head pairs
QC = 512                  # q-chunk width
NQC = S // QC             # 4
NKT = S // 128            # 16 k tiles over sequence
NDT = D // 128            # 8 k tiles over model dim
SCALE = 1.0 / float(np.sqrt(HD))

_CACHE = {}


def _build_nc():
    nc = bacc.Bacc("TRN2", target_bir_lowering=False, debug=False)

    xT = nc.dram_tensor("xT", [D, S], BF16, kind="ExternalInput")
    wqT = nc.dram_tensor("wqT", [D, ELOC], BF16, kind="ExternalInput")
    wkT = nc.dram_tensor("wkT", [D, ELOC], BF16, kind="ExternalInput")
    wvT = nc.dram_tensor("wvT", [D, ELOC], BF16, kind="ExternalInput")
    woT = nc.dram_tensor("woT", [ELOC, D], BF16, kind="ExternalInput")
    bqk = nc.dram_tensor("bqk", [128, 2, NHP], F32, kind="ExternalInput")
    bvb = nc.dram_tensor("bvb", [128, ELOC], F32, kind="ExternalInput")
    outp = nc.dram_tensor("outp", [S, D], F32, kind="ExternalOutput")

    with tile.TileContext(nc) as tc:
        with tc.tile_pool(name="const", bufs=1) as constp, \
             tc.tile_pool(name="wpool", bufs=1) as wp, \
             tc.tile_pool(name="qkv", bufs=1) as qkvp, \
             tc.tile_pool(name="xt", bufs=1) as xtp, \
             tc.tile_pool(name="pt", bufs=12) as ptp, \
             tc.tile_pool(name="otn", bufs=8) as otnp, \
             tc.tile_pool(name="dr", bufs=6) as drp, \
             tc.tile_pool(name="osb", bufs=8) as osbp, \
             tc.tile_pool(name="pss", bufs=2, space="PSUM") as pss, \
             tc.tile_pool(name="psov", bufs=1, space="PSUM") as psov, \
             tc.tile_pool(name="psd", bufs=1, space="PSUM") as psd, \
             tc.tile_pool(name="psx", bufs=2, space="PSUM") as psx:

            # ---- constants ----
            trimask = constp.tile([128, 128], BF16, name="trimask")
            make_upper_triangular(nc, trimask[:], val=1.0, diag=True)
            ones_a = constp.tile([128, 1], BF16, name="ones_a")
            nc.gpsimd.memset(ones_a[:], 1.0)
            ones_b = constp.tile([128, 2], BF16, name="ones_b")
            nc.gpsimd.memset(ones_b[:], 0.0)
            nc.gpsimd.memset(ones_b[:, 0:1], 1.0)
            bc_x = constp.tile([98, 128], BF16, name="bc_x")
            nc.gpsimd.memset(bc_x[:], 0.0)
            nc.gpsimd.memset(bc_x[0:1, 0:64], 1.0)
            nc.gpsimd.memset(bc_x[32:33, 64:128], 1.0)

            bqk_sb = constp.tile([128, 2, NHP], F32, name="bqk_sb")
            nc.sync.dma_start(bqk_sb[:], bqk[:])
            bvb_sb = constp.tile([128, ELOC], F32, name="bvb_sb")
            nc.sync.dma_start(bvb_sb[:], bvb[:])

            # ---- weight + xT DMAs, ordered by first use:
            # wk (KT chunk0), xT (everything), wv (V0), wq (QT c3), wo ----
            wk_sb = []
            for kt in range(NDT):
                t = wp.tile([128, ELOC], BF16, name=f"wk{kt}")
                nc.sync.dma_start(t[:], wkT[128 * kt:128 * (kt + 1), :])
                wk_sb.append(t)
            xts = []
            for kt in range(NDT):
                t = xtp.tile([128, S], BF16, name=f"xt{kt}")
                nc.sync.dma_start(t[:], xT[128 * kt:128 * (kt + 1), :])
                xts.append(t)
            wv_sb = []
            for kt in range(NDT):
                t = wp.tile([128, ELOC], BF16, name=f"wv{kt}")
                nc.sync.dma_start(t[:], wvT[128 * kt:128 * (kt + 1), :])
                wv_sb.append(t)
            wq_sb = []
            for kt in range(NDT):
                t = wp.tile([128, ELOC], BF16, name=f"wq{kt}")
                nc.sync.dma_start(t[:], wqT[128 * kt:128 * (kt + 1), :])
                wq_sb.append(t)
            wo_sb = []
            for hp in range(NHP):
                t = wp.tile([128, D], BF16, name=f"wo{hp}")
                nc.sync.dma_start(t[:], woT[128 * hp:128 * (hp + 1), :])
                wo_sb.append(t)

            # ---- QKV tile holders ----
            QT, KT = [], []
            for hp in range(NHP):
                QT.append(qkvp.tile([128, S], BF16, name=f"qt{hp}"))
                KT.append(qkvp.tile([128, S], BF16, name=f"kt{hp}"))
            V = [qkvp.tile([128, ELOC], BF16, name=f"v{st}") for st in range(NKT)]

            def make_v(st):
                ps = psx.tile([128, ELOC], F32, tag="psx", name="ps_v")
                for kt in range(NDT):
                    nc.tensor.matmul(
                        ps[:],
                        lhsT=xts[kt][:, 128 * st:128 * (st + 1)],
                        rhs=wv_sb[kt][:],
                        start=(kt == 0), stop=(kt == NDT - 1))
                nc.vector.tensor_add(V[st][:], ps[:], bvb_sb[:])

            def make_qk_chunk(which, hp, c):
                dst, wsb, col = ((QT, wq_sb, 0) if which == "q"
                                 else (KT, wk_sb, 1))
                ps = psx.tile([128, QC], F32, tag="psx", name="ps_qk")
                for kt in range(NDT):
                    nc.tensor.matmul(
                        ps[:],
                        lhsT=wsb[kt][:, 128 * hp:128 * (hp + 1)],
                        rhs=xts[kt][:, QC * c:QC * (c + 1)],
                        start=(kt == 0), stop=(kt == NDT - 1))
                nc.vector.tensor_scalar_add(
                    dst[hp][:, QC * c:QC * (c + 1)], ps[:],
                    bqk_sb[:, col, hp:hp + 1])

            # ---- minimal upfront compute: the first attention block
            # (j=3, hp=0) needs KT(hp0) all chunks + QT(hp0) chunk 3;
            # its first AV needs V0/V1 ----
            for c in range(NQC):
                make_qk_chunk("k", 0, c)
            make_qk_chunk("q", 0, 3)
            make_v(0)
            make_v(1)

            # ---- filler queue: each entry is one projection chain (8 MMs
            # + drain).  They are woven into the attention loops at normal
            # priority, in deadline order, so the PE alternates between
            # attention windows and filler chains instead of starving
            # either. ----
            from collections import deque
            fillers = deque()
            for st in range(2, NKT):
                fillers.append(lambda st=st: make_v(st))
            for hp in (1, 2, 3):
                for c in range(NQC):
                    fillers.append(
                        lambda hp=hp, c=c: make_qk_chunk("k", hp, c))
                fillers.append(lambda hp=hp: make_qk_chunk("q", hp, 3))
            for c in (2, 1, 0):
                for hp in (0, 1, 2, 3):
                    fillers.append(
                        lambda hp=hp, c=c: make_qk_chunk("q", hp, c))

            def drain_fillers(n=1):
                for _ in range(min(n, len(fillers))):
                    fillers.popleft()()

            # ---- attention + output projection ----
            tri3 = trimask[:][:, None, :].broadcast_to([128, 2, 128])

            def issue_scores(hp, j, i):
                """Score matmuls for key-tile i of chunk j: 2 MMs, the two
                h2 halves row-pack (lhsT base partitions 0/64)."""
                w = 128 * (i - 4 * j) if i >= 4 * j else 0
                ps_s = pss.tile([128, 2, QC], F32, tag="pss", name="ps_s")
                for h2 in range(2):
                    nc.tensor.matmul(
                        ps_s[:, h2, w:QC],
                        lhsT=KT[hp][64 * h2:64 * (h2 + 1),
                                    128 * i:128 * (i + 1)],
                        rhs=QT[hp][64 * h2:64 * (h2 + 1),
                                   QC * j + w:QC * (j + 1)],
                        start=True, stop=True)
                return ps_s

            for j in (3, 2, 1, 0):
                nkt = 4 * j + 4
                otn_j = {}
                for hp in range(NHP):
                    ps_ot = psov.tile([128, QC], F32, tag="psov",
                                      name="ps_ot")
                    ps_d = psd.tile([128, QC], F32, tag="psd", name="ps_d")
                    if j == 3 and hp == 0:
                        nc.vector.memset(ps_d[:], 1.0)
                    ss = issue_scores(hp, j, 0)
                    pts = {}

                    def issue_av_dens(ii):
                        wd = 128 * (ii - 4 * j) if ii >= 4 * j else 0
                        pd = pts.pop(ii)
                        for h2 in range(2):
                            nc.tensor.matmul(
                                ps_ot[64 * h2:64 * (h2 + 1), wd:QC],
                                lhsT=V[ii][:, 64 * (2 * hp + h2):
                                           64 * (2 * hp + h2 + 1)],
                                rhs=pd[:, h2, wd:QC],
                                start=(ii == 0), stop=(ii == nkt - 1),
                                tile_position=(0, 64 * h2))
                        nc.tensor.matmul(
                            ps_d[0:1, wd:QC],
                            lhsT=ones_a[:], rhs=pd[:, 0, wd:QC],
                            start=(ii == 0), stop=(ii == nkt - 1),
                            tile_position=(0, 0))
                        nc.tensor.matmul(
                            ps_d[32:34, wd:QC],
                            lhsT=ones_b[:], rhs=pd[:, 1, wd:QC],
                            start=(ii == 0), stop=(ii == nkt - 1),
                            tile_position=(0, 32))

                    for i in range(nkt):
                        # keep filler writers traced well ahead of their
                        # readers (V[i] feeds AV(i) two iterations later)
                        drain_fillers(1)
                        w = 128 * (i - 4 * j) if i >= 4 * j else 0
                        last = (i == nkt - 1)
                        pt = ptp.tile([128, 2, QC], BF16, tag="pt",
                                      name="pt")
                        pts[i] = pt
                        nc.scalar.activation(pt[:, :, w:QC], ss[:, :, w:QC],
                                             EXP, scale=SCALE)
                        if i >= 4 * j:
                            nc.vector.tensor_mul(
                                pt[:, :, w:w + 128],
                                pt[:, :, w:w + 128], tri3[:, :, :])
                        if not last:
                            ss = issue_scores(hp, j, i + 1)
                        # AV + denominators lag one iteration: everything
                        # issued here is already dependency-free, so the
                        # PE never waits on the exp just dispatched.
                        if i > 0:
                            issue_av_dens(i - 1)
                            if i % 2 == 0:
                                drain_fillers(1)
                    issue_av_dens(nkt - 1)
                    # normalization: combine the even/odd denominator
                    # accumulators, sanitize + approx reciprocal, then one
                    # broadcast matmul reusing the psd bank.
                    xs = drp.tile([34, QC], F32, name="xs")
                    nc.vector.tensor_scalar_max(xs[:], ps_d[0:34, :],
                                                1e-30)
                    drf = drp.tile([34, QC], F32, name="drf")
                    nc.vector.reciprocal_approx_fast(drf[:], xs[:])
                    dr = drp.tile([34, QC], BF16)
                    with nc.allow_low_precision(reason="denom bf16"):
                        nc.vector.tensor_copy(dr[:], drf[:])
                    nc.tensor.matmul(ps_d[:], lhsT=bc_x[0:34, :],
                                     rhs=dr[:], start=True, stop=True)
                    dbc = drp.tile([128, QC], BF16, name="dbc")
                    nc.vector.tensor_copy(dbc[:], ps_d[:])
                    otn = otnp.tile([128, QC], BF16, tag="otn",
                                    name="otn")
                    nc.vector.tensor_mul(otn[:], ps_ot[:], dbc[:])
                    otn_j[hp] = otn
                # output projection for this q chunk: queued as fillers
                # (it has no downstream consumer besides the final DMA),
                # woven into later blocks' loops.
                def make_outproj(otns, j, m, eo):
                    s0 = QC * j + 128 * m
                    ps_o = psx.tile([128, 512], F32, tag="psx",
                                    name="ps_o")
                    for hp in range(NHP):
                        nc.tensor.matmul(
                            ps_o[:],
                            lhsT=otns[hp][:, 128 * m:128 * (m + 1)],
                            rhs=wo_sb[hp][:, 512 * eo:512 * (eo + 1)],
                            start=(hp == 0), stop=(hp == NHP - 1))
                    osb = osbp.tile([128, 512], F32)
                    if j == 0:
                        # tail chunk: ACT is idle by now — keep DVE free
                        nc.scalar.copy(osb[:], ps_o[:])
                    else:
                        nc.vector.tensor_copy(osb[:], ps_o[:])
                    nc.sync.dma_start(
                        outp[s0:s0 + 128, 512 * eo:512 * (eo + 1)],
                        osb[:])

                otns = dict(otn_j)
                for m in range(4):
                    for eo in range(2):
                        fillers.append(
                            lambda otns=otns, j=j, m=m, eo=eo:
                            make_outproj(otns, j, m, eo))
            drain_fillers(len(fillers))

    nc.compile()
    return nc


def _get_nc():
    if "nc" not in _CACHE:
        _CACHE["nc"] = _build_nc()
    return _CACHE["nc"]


def _prep_core_inputs(x, w_qkv, b_qkv, w_out, b, hg):
    r0 = ELOC * hg
    wq = w_qkv[r0:r0 + ELOC, :]
    wk = w_qkv[D + r0:D + r0 + ELOC, :]
    wv = w_qkv[2 * D + r0:2 * D + r0 + ELOC, :]
    bq = b_qkv[r0:r0 + ELOC]
    bk = b_qkv[D + r0:D + r0 + ELOC]
    bv = b_qkv[2 * D + r0:2 * D + r0 + ELOC]

    bf = ml_dtypes.bfloat16
    bqk_arr = np.empty((128, 2, NHP), np.float32)
    bqk_arr[:, 0, :] = bq.reshape(NHP, 128).T
    bqk_arr[:, 1, :] = bk.reshape(NHP, 128).T
    return {
        "xT": np.ascontiguousarray(x[b].T).astype(bf),
        "wqT": np.ascontiguousarray(wq.T).astype(bf),
        "wkT": np.ascontiguousarray(wk.T).astype(bf),
        "wvT": np.ascontiguousarray(wv.T).astype(bf),
        "woT": np.ascontiguousarray(w_out[:, r0:r0 + ELOC].T).astype(bf),
        "bqk": bqk_arr,
        "bvb": np.tile(bv.astype(np.float32)[None, :], (128, 1)),
    }


def kernel(x, w_qkv, b_qkv, w_out, b_out, _trace=False, _trace_kwargs=None):
    x = np.asarray(x, np.float32)
    w_qkv = np.asarray(w_qkv, np.float32)
    b_qkv = np.asarray(b_qkv, np.float32)
    w_out = np.asarray(w_out, np.float32)
    b_out = np.asarray(b_out, np.float32)

    nc = _get_nc()
    in_maps = []
    for core in range(NCORES):
        b, hg = core // 2, core % 2
        in_maps.append(_prep_core_inputs(x, w_qkv, b_qkv, w_out, b, hg))

    kw = {}
    if _trace:
        kw.update(trace=True, **(_trace_kwargs or {}))
    import time
    res = None
    for attempt in range(4):
        try:
            res = bass_utils.run_bass_kernel_spmd(
                nc, in_maps, core_ids=list(range(NCORES)), **kw)
            break
        except Exception:
            if attempt == 3:
                raise
            # Transient axon/NRT device flake: reset the PJRT backend so the
            # retry starts from a clean client, like a fresh process would.
            try:
                import jax
                jax.clear_caches()
                import jax._src.xla_bridge as _xb
                _xb._clear_backends()
            except Exception:
                pass
            time.sleep(5.0 * (attempt + 1))

    out = np.empty((B, S, D), np.float32)
    for b in range(B):
        out[b] = res.results[2 * b]["outp"] + res.results[2 * b + 1]["outp"] \
            + b_out[None, :]
    if _trace:
        return out, res
    return out


# revision 37
# speedup vs baseline: 1.0654x; 1.0105x over previous
"""Causal multi-head attention (dense transformer block) on 8 Trainium2 cores.

Problem: x[4, 2048, 1024], 16 heads, head_dim 64, causal softmax attention
with QKV + output projections (torch Linear layout weights).

Sharding: 8 cores = 4 batches x 2 head-groups (8 heads each).  Each core
computes QKV projection for its 8 heads, attention, and its partial output
projection (row-parallel over w_out).  Host sums the two partials per batch
and adds b_out.

All device layouts are "transposed" so no on-device transposes are needed:
  - x is fed as xT [d, s]; Q^T/K^T are produced as [head_dim, s]
  - scores are computed as S^T [k, q]; softmax runs along partitions via
    ones-matmul column sums; O is accumulated directly as O^T [e_loc, q],
    which is exactly the lhsT the output projection needs.
Matmul inputs are bf16 (PSUM accumulation is fp32); everything else fp32.

Schedule: attention processes one head-pair at a time (psov needs only one
PSUM bank, freeing a second bank so projection/output chains double-buffer
through the shared "psx" pool).  Only the minimum QKV slice runs up front
(KT/QT(hp0) + V0/V1); the remaining V/QT/KT chunks and the output
projection live in a deadline-ordered filler queue drained inside the
attention i-loops, so each writer is traced before its readers (Tile
tracks RAW deps by trace order) while the PE interleaves chains into exp
gaps.  Within the i-loop, scores for tile i+1 are issued before AV(i) so
ACT's exp stream never starves.

Note: PSUM accumulation-group starts (start=True) clear has_written for
the whole bank, so the per-(j,hp) denominator accumulator keeps a single
group (rows 0/32) — interleaving a second group's start on the same bank
corrupts or races.
"""

import sys

sys.path.insert(0, "/opt/trn_rl_repo")

import numpy as np
import ml_dtypes

import concourse.bass as bass
import concourse.mybir as mybir
import concourse.tile as tile
from concourse import bacc
from concourse import bass_utils
from concourse.masks import make_upper_triangular

F32 = mybir.dt.float32
BF16 = mybir.dt.bfloat16
EXP = mybir.ActivationFunctionType.Exp

B, S, D = 4, 2048, 1024
HTOT, HD = 16, 64
NCORES = 8
HLOC = HTOT // 2          # heads per core
ELOC = HLOC * HD          # 512 local embedding width
NHP = HLOC // 2           # 4 head pairs
QC = 512                  # q-chunk width
NQC = S // QC             # 4
NKT = S // 128            # 16 k tiles over sequence
NDT = D // 128            # 8 k tiles over model dim
SCALE = 1.0 / float(np.sqrt(HD))

_CACHE = {}


def _build_nc():
    nc = bacc.Bacc("TRN2", target_bir_lowering=False, debug=False)

    xT = nc.dram_tensor("xT", [D, S], BF16, kind="ExternalInput")
    wqT = nc.dram_tensor("wqT", [D, ELOC], BF16, kind="ExternalInput")
    wkT = nc.dram_tensor("wkT", [D, ELOC], BF16, kind="ExternalInput")
    wvT = nc.dram_tensor("wvT", [D, ELOC], BF16, kind="ExternalInput")
    woT = nc.dram_tensor("woT", [ELOC, D], BF16, kind="ExternalInput")
    bqk = nc.dram_tensor("bqk", [128, 2, NHP], F32, kind="ExternalInput")
    bvb = nc.dram_tensor("bvb", [128, ELOC], F32, kind="ExternalInput")
    outp = nc.dram_tensor("outp", [S, D], F32, kind="ExternalOutput")

    with tile.TileContext(nc) as tc:
        with tc.tile_pool(name="const", bufs=1) as constp, \
             tc.tile_pool(name="wpool", bufs=1) as wp, \
             tc.tile_pool(name="qkv", bufs=1) as qkvp, \
             tc.tile_pool(name="xt", bufs=1) as xtp, \
             tc.tile_pool(name="pt", bufs=12) as ptp, \
             tc.tile_pool(name="otn", bufs=8) as otnp, \
             tc.tile_pool(name="dr", bufs=4) as drp, \
             tc.tile_pool(name="osb", bufs=6) as osbp, \
             tc.tile_pool(name="pss", bufs=2, space="PSUM") as pss, \
             tc.tile_pool(name="psov", bufs=1, space="PSUM") as psov, \
             tc.tile_pool(name="psd", bufs=1, space="PSUM") as psd, \
             tc.tile_pool(name="psx", bufs=2, space="PSUM") as psx:

            # ---- constants ----
            trimask = constp.tile([128, 128], BF16, name="trimask")
            make_upper_triangular(nc, trimask[:], val=1.0, diag=True)
            ones_a = constp.tile([128, 1], BF16, name="ones_a")
            nc.gpsimd.memset(ones_a[:], 1.0)
            ones_b = constp.tile([128, 2], BF16, name="ones_b")
            nc.gpsimd.memset(ones_b[:], 0.0)
            nc.gpsimd.memset(ones_b[:, 0:1], 1.0)
            bc_x = constp.tile([98, 128], BF16, name="bc_x")
            nc.gpsimd.memset(bc_x[:], 0.0)
            nc.gpsimd.memset(bc_x[0:1, 0:64], 1.0)
            nc.gpsimd.memset(bc_x[32:33, 64:128], 1.0)

            bqk_sb = constp.tile([128, 2, NHP], F32, name="bqk_sb")
            nc.sync.dma_start(bqk_sb[:], bqk[:])
            bvb_sb = constp.tile([128, ELOC], F32, name="bvb_sb")
            nc.sync.dma_start(bvb_sb[:], bvb[:])

            # ---- weight + xT DMAs, ordered by first use:
            # wk (KT chunk0), xT (everything), wv (V0), wq (QT c3), wo ----
            wk_sb = []
            for kt in range(NDT):
                t = wp.tile([128, ELOC], BF16, name=f"wk{kt}")
                nc.sync.dma_start(t[:], wkT[128 * kt:128 * (kt + 1), :])
                wk_sb.append(t)
            xts = []
            for kt in range(NDT):
                t = xtp.tile([128, S], BF16, name=f"xt{kt}")
                nc.sync.dma_start(t[:], xT[128 * kt:128 * (kt + 1), :])
                xts.append(t)
            wv_sb = []
            for kt in range(NDT):
                t = wp.tile([128, ELOC], BF16, name=f"wv{kt}")
                nc.sync.dma_start(t[:], wvT[128 * kt:128 * (kt + 1), :])
                wv_sb.append(t)
            wq_sb = []
            for kt in range(NDT):
                t = wp.tile([128, ELOC], BF16, name=f"wq{kt}")
                nc.sync.dma_start(t[:], wqT[128 * kt:128 * (kt + 1), :])
                wq_sb.append(t)
            wo_sb = []
            for hp in range(NHP):
                t = wp.tile([128, D], BF16, name=f"wo{hp}")
                nc.sync.dma_start(t[:], woT[128 * hp:128 * (hp + 1), :])
                wo_sb.append(t)

            # ---- QKV tile holders ----
            QT, KT = [], []
            for hp in range(NHP):
                QT.append(qkvp.tile([128, S], BF16, name=f"qt{hp}"))
                KT.append(qkvp.tile([128, S], BF16, name=f"kt{hp}"))
            V = [qkvp.tile([128, ELOC], BF16, name=f"v{st}") for st in range(NKT)]

            def make_v(st):
                ps = psx.tile([128, ELOC], F32, tag="psx", name="ps_v")
                for kt in range(NDT):
                    nc.tensor.matmul(
                        ps[:],
                        lhsT=xts[kt][:, 128 * st:128 * (st + 1)],
                        rhs=wv_sb[kt][:],
                        start=(kt == 0), stop=(kt == NDT - 1))
                nc.vector.tensor_add(V[st][:], ps[:], bvb_sb[:])

            def make_qk_chunk(which, hp, c):
                dst, wsb, col = ((QT, wq_sb, 0) if which == "q"
                                 else (KT, wk_sb, 1))
                ps = psx.tile([128, QC], F32, tag="psx", name="ps_qk")
                for kt in range(NDT):
                    nc.tensor.matmul(
                        ps[:],
                        lhsT=wsb[kt][:, 128 * hp:128 * (hp + 1)],
                        rhs=xts[kt][:, QC * c:QC * (c + 1)],
                        start=(kt == 0), stop=(kt == NDT - 1))
                nc.vector.tensor_scalar_add(
                    dst[hp][:, QC * c:QC * (c + 1)], ps[:],
                    bqk_sb[:, col, hp:hp + 1])

            # ---- minimal upfront compute: the first attention block
            # (j=3, hp=0) needs KT(hp0) all chunks + QT(hp0) chunk 3;
            # its first AV needs V0/V1 ----
            for c in range(NQC):
                make_qk_chunk("k", 0, c)
            make_qk_chunk("q", 0, 3)
            make_v(0)
            make_v(1)

            # ---- filler queue: each entry is one projection chain (8 MMs
            # + drain).  They are woven into the attention loops at normal
            # priority, in deadline order, so the PE alternates between
            # attention windows and filler chains instead of starving
            # either. ----
            from collections import deque
            fillers = deque()
            for st in range(2, NKT):
                fillers.append(lambda st=st: make_v(st))
            for hp in (1, 2, 3):
                for c in range(NQC):
                    fillers.append(
                        lambda hp=hp, c=c: make_qk_chunk("k", hp, c))
                fillers.append(lambda hp=hp: make_qk_chunk("q", hp, 3))
            for c in (2, 1, 0):
                for hp in (0, 1, 2, 3):
                    fillers.append(
                        lambda hp=hp, c=c: make_qk_chunk("q", hp, c))

            def drain_fillers(n=1):
                for _ in range(min(n, len(fillers))):
                    fillers.popleft()()

            # ---- attention + output projection ----
            tri3 = trimask[:][:, None, :].broadcast_to([128, 2, 128])

            def issue_scores(hp, j, i):
                """Score matmuls for key-tile i of chunk j: 2 MMs, the two
                h2 halves row-pack (lhsT base partitions 0/64)."""
                w = 128 * (i - 4 * j) if i >= 4 * j else 0
                ps_s = pss.tile([128, 2, QC], F32, tag="pss", name="ps_s")
                for h2 in range(2):
                    nc.tensor.matmul(
                        ps_s[:, h2, w:QC],
                        lhsT=KT[hp][64 * h2:64 * (h2 + 1),
                                    128 * i:128 * (i + 1)],
                        rhs=QT[hp][64 * h2:64 * (h2 + 1),
                                   QC * j + w:QC * (j + 1)],
                        start=True, stop=True)
                return ps_s

            for j in (3, 2, 1, 0):
                nkt = 4 * j + 4
                otn_j = {}
                for hp in range(NHP):
                    ps_ot = psov.tile([128, QC], F32, tag="psov",
                                      name="ps_ot")
                    ps_d = psd.tile([128, QC], F32, tag="psd", name="ps_d")
                    if j == 3 and hp == 0:
                        nc.vector.memset(ps_d[:], 1.0)
                    ss = issue_scores(hp, j, 0)
                    pts = {}

                    def issue_av_dens(ii):
                        wd = 128 * (ii - 4 * j) if ii >= 4 * j else 0
                        pd = pts.pop(ii)
                        for h2 in range(2):
                            nc.tensor.matmul(
                                ps_ot[64 * h2:64 * (h2 + 1), wd:QC],
                                lhsT=V[ii][:, 64 * (2 * hp + h2):
                                           64 * (2 * hp + h2 + 1)],
                                rhs=pd[:, h2, wd:QC],
                                start=(ii == 0), stop=(ii == nkt - 1),
                                tile_position=(0, 64 * h2))
                        nc.tensor.matmul(
                            ps_d[0:1, wd:QC],
                            lhsT=ones_a[:], rhs=pd[:, 0, wd:QC],
                            start=(ii == 0), stop=(ii == nkt - 1),
                            tile_position=(0, 0))
                        nc.tensor.matmul(
                            ps_d[32:34, wd:QC],
                            lhsT=ones_b[:], rhs=pd[:, 1, wd:QC],
                            start=(ii == 0), stop=(ii == nkt - 1),
                            tile_position=(0, 32))

                    for i in range(nkt):
                        # keep filler writers traced well ahead of their
                        # readers (V[i] feeds AV(i) two iterations later)
                        drain_fillers(1)
                        w = 128 * (i - 4 * j) if i >= 4 * j else 0
                        last = (i == nkt - 1)
                        pt = ptp.tile([128, 2, QC], BF16, tag="pt",
                                      name="pt")
                        pts[i] = pt
                        nc.scalar.activation(pt[:, :, w:QC], ss[:, :, w:QC],
                                             EXP, scale=SCALE)
                        if i >= 4 * j:
                            nc.vector.tensor_mul(
                                pt[:, :, w:w + 128],
                                pt[:, :, w:w + 128], tri3[:, :, :])
                        if not last:
                            ss = issue_scores(hp, j, i + 1)
                        # AV + denominators lag one iteration: everything
                        # issued here is already dependency-free, so the
                        # PE never waits on the exp just dispatched.
                        if i > 0:
                            issue_av_dens(i - 1)
                            if i % 2 == 0:
                                drain_fillers(1)
                    issue_av_dens(nkt - 1)
                    # normalization: combine the even/odd denominator
                    # accumulators, sanitize + approx reciprocal, then one
                    # broadcast matmul reusing the psd bank.
                    xs = drp.tile([34, QC], F32, name="xs")
                    nc.vector.tensor_scalar_max(xs[:], ps_d[0:34, :],
                                                1e-30)
                    drf = drp.tile([34, QC], F32, name="drf")
                    nc.vector.reciprocal_approx_fast(drf[:], xs[:])
                    dr = drp.tile([34, QC], BF16)
                    with nc.allow_low_precision(reason="denom bf16"):
                        nc.vector.tensor_copy(dr[:], drf[:])
                    nc.tensor.matmul(ps_d[:], lhsT=bc_x[0:34, :],
                                     rhs=dr[:], start=True, stop=True)
                    dbc = drp.tile([128, QC], BF16, name="dbc")
                    nc.vector.tensor_copy(dbc[:], ps_d[:])
                    otn = otnp.tile([128, QC], BF16, tag="otn",
                                    name="otn")
                    nc.vector.tensor_mul(otn[:], ps_ot[:], dbc[:])
                    otn_j[hp] = otn
                # output projection for this q chunk: queued as fillers
                # (it has no downstream consumer besides the final DMA),
                # woven into later blocks' loops.
                def make_outproj(otns, j, m, eo):
                    s0 = QC * j + 128 * m
                    ps_o = psx.tile([128, 512], F32, tag="psx",
                                    name="ps_o")
                    for hp in range(NHP):
                        nc.tensor.matmul(
                            ps_o[:],
                            lhsT=otns[hp][:, 128 * m:128 * (m + 1)],
                            rhs=wo_sb[hp][:, 512 * eo:512 * (eo + 1)],
                            start=(hp == 0), stop=(hp == NHP - 1))
                    osb = osbp.tile([128, 512], F32)
                    nc.vector.tensor_copy(osb[:], ps_o[:])
                    nc.sync.dma_start(
                        outp[s0:s0 + 128, 512 * eo:512 * (eo + 1)],
                        osb[:])

                otns = dict(otn_j)
                for m in range(4):
                    for eo in range(2):
                        fillers.append(
                            lambda otns=otns, j=j, m=m, eo=eo:
                            make_outproj(otns, j, m, eo))
            drain_fillers(len(fillers))

    nc.compile()
    return nc


def _get_nc():
    if "nc" not in _CACHE:
        _CACHE["nc"] = _build_nc()
    return _CACHE["nc"]


def _prep_core_inputs(x, w_qkv, b_qkv, w_out, b, hg):
    r0 = ELOC * hg
    wq = w_qkv[r0:r0 + ELOC, :]
    wk = w_qkv[D + r0:D + r0 + ELOC, :]
    wv = w_qkv[2 * D + r0:2 * D + r0 + ELOC, :]
    bq = b_qkv[r0:r0 + ELOC]
    bk = b_qkv[D + r0:D + r0 + ELOC]
    bv = b_qkv[2 * D + r0:2 * D + r0 + ELOC]

    bf = ml_dtypes.bfloat16
    bqk_arr = np.empty((128, 2, NHP), np.float32)
    bqk_arr[:, 0, :] = bq.reshape(NHP, 128).T
    bqk_arr[:, 1, :] = bk.reshape(NHP, 128).T
    return {
        "xT": np.ascontiguousarray(x[b].T).astype(bf),
        "wqT": np.ascontiguousarray(wq.T).astype(bf),
        "wkT": np.ascontiguousarray(wk.T).astype(bf),
        "wvT": np.ascontiguousarray(wv.T).astype(bf),
        "woT": np.ascontiguousarray(w_out[:, r0:r0 + ELOC].T).astype(bf),
        "bqk": bqk_arr,
        "bvb": np.tile(bv.astype(np.float32)[None, :], (128, 1)),
    }


def kernel(x, w_qkv, b_qkv, w_out, b_out, _trace=False, _trace_kwargs=None):
    x = np.asarray(x, np.float32)
    w_qkv = np.asarray(w_qkv, np.float32)
    b_qkv = np.asarray(b_qkv, np.float32)
    w_out = np.asarray(w_out, np.float32)
    b_out = np.asarray(b_out, np.float32)

    nc = _get_nc()
    in_maps = []
    for core in range(NCORES):
        b, hg = core // 2, core % 2
        in_maps.append(_prep_core_inputs(x, w_qkv, b_qkv, w_out, b, hg))

    kw = {}
    if _trace:
        kw.update(trace=True, **(_trace_kwargs or {}))
    import time
    res = None
    for attempt in range(4):
        try:
            res = bass_utils.run_bass_kernel_spmd(
                nc, in_maps, core_ids=list(range(NCORES)), **kw)
            break
        except Exception:
            if attempt == 3:
                raise
            # Transient axon/NRT device flake: reset the PJRT backend so the
            # retry starts from a clean client, like a fresh process would.
            try:
                import jax
                jax.clear_caches()
                import jax._src.xla_bridge as _xb
                _xb._clear_backends()
            except Exception:
                pass
            time.sleep(5.0 * (attempt + 1))

    out = np.empty((B, S, D), np.float32)
    for b in range(B):
        out[b] = res.results[2 * b]["outp"] + res.results[2 * b + 1]["outp"] \
            + b_out[None, :]
    if _trace:
        return out, res
    return out
